# revision 8
# baseline (speedup 1.0000x reference)
"""Trainium2 Bass kernel for nn_Kernel3D (Gaussian splat onto a 64x64x64x8 grid).

Math:  out[x,y,z,t] = sum_n bx[n,x] * by[n,y] * bz[n,z] * x[n,t]
where b?[n,g] = exp(-0.5*((g-mu)/s)^2) / sqrt(2*pi*s^2)  (normalized Gaussian basis).

Strategy: shard the output X dimension across the 8 cores (8 x-planes each).
Per core the computation is one dense matmul
    out[(x y), (t z)] = P[n, (x y)]^T @ Q[n, (t z)]
with P[n, x*64+y] = bx[n,x]*by[n,y] (built as exp(-0.5*(ux^2+uy^2)) on chip)
and  Q[n, t*64+z] = (x[n,t]*Cn) * bz[n,z], Cn = (2*pi)^-1.5/(sx*sy*sz).
Contraction over n runs in chunks of 128 points (PSUM accumulation).
Each core only needs the points whose x-Gaussian overlaps its 8-voxel slab,
so points are binned per core host-side (pure sharding, no host math on values).
"""

import os
import sys

import numpy as np

for _p in ("/opt/trn_rl_repo", "/root/.axon_site/_ro/trn_rl_repo"):
    if os.path.isdir(_p) and _p not in sys.path:
        sys.path.insert(0, _p)

N_CORES = 8
GX, GY, GZ, GT = 64, 64, 64, 8
XPER = GX // N_CORES  # x-planes per core
PPC = 128  # points per chunk (partition dim)
FEAT = 16  # packed per-point features: x[8], mu[3], sigma[3], pad[2]

# Point selection: keep a point for a core if its x-Gaussian reaches the
# core's slab within SIGMA_CUT sigmas. exp(-0.5*4.5^2) ~ 4e-5 -> negligible.
SIGMA_CUT = 4.5
SELECT_POINTS = True

MM_DTYPE = "float32r"  # matmul input dtype: float32r = 1 cycle/row on trn2

_prog_cache = {}


def _build(n_chunks, mm_dt_name):
    import concourse.bass as bass
    import concourse.tile as tile
    from concourse import mybir
    from contextlib import ExitStack

    f32 = mybir.dt.float32
    mm_dt = getattr(mybir.dt, mm_dt_name)
    AL = mybir.AluOpType
    ACTF = mybir.ActivationFunctionType
    C0 = float((2.0 * np.pi) ** -1.5)

    nc = bass.Bass(use_seq_codegen=True)
    pts = nc.declare_dram_parameter("pts", [PPC, n_chunks * FEAT], f32, isOutput=False)
    xgrid = nc.declare_dram_parameter("xgrid", [PPC, XPER], f32, isOutput=False)
    iotayz = nc.declare_dram_parameter("iotayz", [PPC, GY], f32, isOutput=False)
    out = nc.declare_dram_parameter("out", [XPER * GY, GT * GZ], f32, isOutput=True)

    with tile.TileContext(nc) as tc, ExitStack() as ctx:
        cpool = ctx.enter_context(tc.tile_pool(name="const", bufs=1))
        wpool = ctx.enter_context(tc.tile_pool(name="work", bufs=3))
        opool = ctx.enter_context(tc.tile_pool(name="outp", bufs=2))
        ppool = ctx.enter_context(tc.tile_pool(name="accp", bufs=1, space="PSUM"))

        pts_t = cpool.tile([PPC, n_chunks * FEAT], f32, name="pts_t")
        nc.sync.dma_start(pts_t[:, :], pts[:, :])
        xg_t = cpool.tile([PPC, XPER], f32, name="xg_t")
        nc.sync.dma_start(xg_t[:, :], xgrid[:, :])
        io_t = cpool.tile([PPC, GY], f32, name="io_t")
        nc.sync.dma_start(io_t[:, :], iotayz[:, :])

        pts3 = pts_t[:, :].rearrange("p (c f) -> p c f", f=FEAT)

        # Batched per-point scalars for all chunks at once:
        #   inv_s = 1/sigma;  m2 = C0/(sx*sy*sz);  xc[n,t] = x[n,t]*m2[n]
        inv_t = cpool.tile([PPC, n_chunks, 3], f32, name="inv_t")
        nc.vector.reciprocal(inv_t[:, :, :], pts3[:, :, 11:14])
        m1_t = cpool.tile([PPC, n_chunks], f32, name="m1_t")
        nc.vector.tensor_tensor(m1_t[:, :], inv_t[:, :, 0], inv_t[:, :, 1], AL.mult)
        m2_t = cpool.tile([PPC, n_chunks], f32, name="m2_t")
        nc.vector.scalar_tensor_tensor(
            m2_t[:, :], m1_t[:, :], C0, inv_t[:, :, 2], AL.mult, AL.mult
        )
        xc_t = cpool.tile([PPC, n_chunks, GT], f32, name="xc_t")
        nc.vector.tensor_tensor(
            xc_t[:, :, :],
            pts3[:, :, 0:GT],
            m2_t[:, :].unsqueeze(2).broadcast_to((PPC, n_chunks, GT)),
            AL.mult,
        )

        accs = [
            ppool.tile([128, 512], f32, tag=f"acc{m}", name=f"acc{m}") for m in range(4)
        ]

        for c in range(n_chunks):
            mu_x = pts3[:, c, 8:9]
            mu_y = pts3[:, c, 9:10]
            mu_z = pts3[:, c, 10:11]
            ivx = inv_t[:, c, 0:1]
            ivy = inv_t[:, c, 1:2]
            ivz = inv_t[:, c, 2:3]

            # u = [(xg-mux)/sx | (yg-muy)/sy | (zg-muz)/sz], 136 wide, on DVE
            u_t = wpool.tile([PPC, 136], f32, name="u_t", tag="ubuf")
            nc.vector.scalar_tensor_tensor(
                u_t[:, 0:8], xg_t[:, :], mu_x, ivx.broadcast_to((PPC, XPER)),
                AL.subtract, AL.mult,
            )
            nc.vector.scalar_tensor_tensor(
                u_t[:, 8:72], io_t[:, :], mu_y, ivy.broadcast_to((PPC, GY)),
                AL.subtract, AL.mult,
            )
            nc.vector.scalar_tensor_tensor(
                u_t[:, 72:136], io_t[:, :], mu_z, ivz.broadcast_to((PPC, GZ)),
                AL.subtract, AL.mult,
            )
            # b = exp(-0.5*u^2): square then exp, both on ACT (single producer)
            sq_t = wpool.tile([PPC, 136], f32, name="sq_t", tag="sqbuf")
            nc.scalar.activation(sq_t[:, :], u_t[:, :], ACTF.Square)
            b_t = wpool.tile([PPC, 136], f32, name="b_t", tag="bbuf")
            nc.scalar.activation(b_t[:, :], sq_t[:, :], ACTF.Exp, scale=-0.5)

            # P[n, j*64+y] = bx[n,j]*by[n,y];  Q[n, t*64+z] = xc[n,t]*bz[n,z]
            # both built on DVE so the matmul has a single producer engine
            p_t = wpool.tile([PPC, 512], mm_dt, name="p_t", tag="pbuf")
            nc.vector.tensor_tensor(
                p_t[:, :].rearrange("p (a b) -> p a b", b=GY),
                b_t[:, 0:8].unsqueeze(2).broadcast_to((PPC, XPER, GY)),
                b_t[:, 8:72].unsqueeze(1).broadcast_to((PPC, XPER, GY)),
                AL.mult,
            )
            q_t = wpool.tile([PPC, 512], mm_dt, name="q_t", tag="qbuf")
            nc.vector.tensor_tensor(
                q_t[:, :].rearrange("p (a b) -> p a b", b=GZ),
                xc_t[:, c, :].unsqueeze(2).broadcast_to((PPC, GT, GZ)),
                b_t[:, 72:136].unsqueeze(1).broadcast_to((PPC, GT, GZ)),
                AL.mult,
            )

            for m in range(4):
                nc.tensor.matmul(
                    accs[m][:, :],
                    lhsT=p_t[:, m * 128 : (m + 1) * 128],
                    rhs=q_t[:, :],
                    start=(c == 0),
                    stop=(c == n_chunks - 1),
                )

        for m in range(4):
            o_t = opool.tile([128, 512], f32, name="o_t", tag="obuf")
            nc.scalar.copy(o_t[:, :], accs[m][:, :])
            nc.sync.dma_start(out[m * 128 : (m + 1) * 128, :], o_t[:, :])

    _split_multi_waits(nc, mybir)
    return nc


def _split_multi_waits(nc, mybir):
    """This walrus build rejects instructions carrying >1 sync-wait command.
    Hoist extra waits onto standalone same-engine InstEventSemaphore
    instructions inserted immediately before the overloaded instruction —
    identical semantics (sequencer blocks on each wait in program order)."""
    k = 0
    for bb in nc.m.functions[0].blocks:
        new = []
        for inst in bb.instructions:
            si = inst.sync_info
            if si is not None and si.on_wait and len(si.on_wait) > 1:
                for w in si.on_wait[:-1]:
                    wi = mybir.InstEventSemaphore(
                        name=f"wsplit_{k}", ins=[], outs=[]
                    )
                    k += 1
                    wi.engine = inst.engine
                    wi.sync_info = mybir.SyncInfo(on_wait=[w], on_update=[])
                    nc.register_instruction(wi)
                    new.append(wi)
                inst.sync_info = mybir.SyncInfo(
                    on_wait=[si.on_wait[-1]], on_update=si.on_update
                )
            new.append(inst)
        bb.instructions[:] = new


def _get_prog(n_chunks, mm_dt_name):
    key = (n_chunks, mm_dt_name)
    if key not in _prog_cache:
        _prog_cache[key] = _build(n_chunks, mm_dt_name)
    return _prog_cache[key]


def _pack_points(x, mu, sigma, n_chunks):
    """[n,8]+[n,3]+[n,3] -> [128, n_chunks*16] chunk-packed layout.

    Padding rows use sigma=1 / x=0 so they contribute exactly zero and
    produce no NaN/Inf anywhere in the pipeline.
    """
    n = x.shape[0]
    cap = n_chunks * PPC
    feat = np.zeros((cap, FEAT), np.float32)
    feat[:, 11:14] = 1.0  # sigma=1 for padding rows
    feat[:n, 0:8] = x
    feat[:n, 8:11] = mu
    feat[:n, 11:14] = sigma
    return (
        feat.reshape(n_chunks, PPC, FEAT).transpose(1, 0, 2).reshape(PPC, n_chunks * FEAT)
    )


def _prepare(x, mu, sigma):
    n = x.shape[0]
    if SELECT_POINTS:
        sel = []
        for c in range(N_CORES):
            lo, hi = c * XPER, c * XPER + XPER - 1  # inclusive grid range
            d = np.maximum.reduce([lo - mu[:, 0], mu[:, 0] - hi, np.zeros(n, np.float32)])
            sel.append(np.nonzero(d <= SIGMA_CUT * sigma[:, 0])[0])
        n_chunks = max(1, int(np.ceil(max(len(s) for s in sel) / PPC)))
    else:
        sel = [np.arange(n) for _ in range(N_CORES)]
        n_chunks = (n + PPC - 1) // PPC

    iota = np.tile(np.arange(GY, dtype=np.float32), (PPC, 1))
    in_maps = []
    for c in range(N_CORES):
        idx = sel[c]
        in_maps.append(
            {
                "pts": _pack_points(x[idx], mu[idx], sigma[idx], n_chunks),
                "xgrid": np.tile(
                    np.arange(c * XPER, (c + 1) * XPER, dtype=np.float32), (PPC, 1)
                ),
                "iotayz": iota,
            }
        )
    return in_maps, n_chunks


def _assemble(results):
    o = np.stack([results[c]["out"] for c in range(N_CORES)])  # [8, 512, 512]
    o = o.reshape(N_CORES, XPER, GY, GT, GZ).transpose(0, 1, 2, 4, 3)
    return np.ascontiguousarray(o.reshape(GX, GY, GZ, GT))


def run(x, mu, sigma, trace=False, **spmd_kwargs):
    """Returns (output, BassKernelResults)."""
    from concourse.bass_utils import run_bass_kernel_spmd

    x = np.asarray(x, np.float32)
    mu = np.asarray(mu, np.float32)
    sigma = np.asarray(sigma, np.float32)
    in_maps, n_chunks = _prepare(x, mu, sigma)
    nc = _get_prog(n_chunks, MM_DTYPE)
    res = run_bass_kernel_spmd(
        nc, in_maps, list(range(N_CORES)), trace=trace, **spmd_kwargs
    )
    return _assemble(res.results), res


def kernel(x, mu, sigma):
    out, _ = run(x, mu, sigma)
    return out
